# revision 1
# baseline (speedup 1.0000x reference)
import numpy as np

B, T, H, L = 64, 2048, 256, 16
NCORES = 8
BS = B // NCORES          # 8 sequences per core
PTS = BS * T              # 16384 points per core, column index = t*BS + b
RENORM = 8


def _build_nc():
    import concourse.bass as bass
    import concourse.mybir as mybir
    from concourse.tile import TileContext

    f32 = mybir.dt.float32
    nc = bass.Bass()

    xt = nc.dram_tensor("xt", [H, PTS], f32, kind="ExternalInput")
    wt = nc.dram_tensor("wt", [H, L], f32, kind="ExternalInput")
    expT = nc.dram_tensor("expT", [L, L], f32, kind="ExternalInput")
    estart = nc.dram_tensor("estart", [L, 1], f32, kind="ExternalInput")
    eend = nc.dram_tensor("eend", [L, 1], f32, kind="ExternalInput")
    ones16 = nc.dram_tensor("ones16", [L, L], f32, kind="ExternalInput")
    em_out = nc.dram_tensor("em_out", [L, PTS], f32, kind="ExternalOutput")
    den_out = nc.dram_tensor("den_out", [1, BS], f32, kind="ExternalOutput")

    CH = 512
    NCH = PTS // CH
    EXP = mybir.ActivationFunctionType.Exp
    LN = mybir.ActivationFunctionType.Ln

    with TileContext(nc) as tc:
        with (
            tc.tile_pool(name="singles", bufs=1) as singles,
            tc.tile_pool(name="xtiles", bufs=3) as xtiles,
            tc.tile_pool(name="empsum", bufs=2, space="PSUM") as empsum,
            tc.tile_pool(name="scan", bufs=3) as scan,
            tc.tile_pool(name="scanp", bufs=4, space="PSUM") as scanp,
        ):
            wt0 = singles.tile([128, L], f32, tag="wt0")
            wt1 = singles.tile([128, L], f32, tag="wt1")
            expT_sb = singles.tile([L, L], f32, tag="expT")
            estart_sb = singles.tile([L, 1], f32, tag="estart")
            eend_sb = singles.tile([L, 1], f32, tag="eend")
            ones_sb = singles.tile([L, L], f32, tag="ones")
            em_sb = singles.tile([L, PTS], f32, tag="em")
            eem_sb = singles.tile([L, PTS], f32, tag="eem")
            logz = singles.tile([1, BS], f32, tag="logz")
            den_sb = singles.tile([1, BS], f32, tag="den")

            nc.sync.dma_start(wt0, wt[0:128, :])
            nc.sync.dma_start(wt1, wt[128:256, :])
            nc.sync.dma_start(expT_sb, expT[:, :])
            nc.sync.dma_start(estart_sb, estart[:, :])
            nc.sync.dma_start(eend_sb, eend[:, :])
            nc.sync.dma_start(ones_sb, ones16[:, :])
            nc.any.memzero(logz)

            # emissions^T = W @ x^T  (K=H contracted in two 128-chunks)
            for c in range(NCH):
                x0 = xtiles.tile([128, CH], f32, tag="x0")
                x1 = xtiles.tile([128, CH], f32, tag="x1")
                nc.sync.dma_start(x0, xt[0:128, c * CH:(c + 1) * CH])
                nc.sync.dma_start(x1, xt[128:256, c * CH:(c + 1) * CH])
                ps = empsum.tile([L, CH], f32, tag="emps")
                nc.tensor.matmul(ps, wt0, x0, start=True, stop=False)
                nc.tensor.matmul(ps, wt1, x1, start=False, stop=True)
                nc.any.tensor_copy(em_sb[:, c * CH:(c + 1) * CH], ps)
                nc.scalar.activation(eem_sb[:, c * CH:(c + 1) * CH], ps, EXP)

            nc.sync.dma_start(em_out[:, :], em_sb)

            # forward scan in scaled-exp domain, aT[i, b]
            aT = scan.tile([L, BS], f32, tag="aT")
            nc.any.tensor_scalar_mul(aT, eem_sb[:, 0:BS], estart_sb)
            for t in range(1, T):
                ps = scanp.tile([L, BS], f32, tag="sps")
                nc.tensor.matmul(ps, expT_sb, aT, start=True, stop=True)
                aT = scan.tile([L, BS], f32, tag="aT")
                nc.vector.tensor_mul(aT, ps, eem_sb[:, t * BS:(t + 1) * BS])
                if t % RENORM == 0:
                    cs = scanp.tile([L, BS], f32, tag="cs")
                    nc.tensor.matmul(cs, ones_sb, aT, start=True, stop=True)
                    rec = scan.tile([L, BS], f32, tag="rec")
                    nc.vector.reciprocal(rec, cs)
                    aT2 = scan.tile([L, BS], f32, tag="aT")
                    nc.vector.tensor_mul(aT2, aT, rec)
                    aT = aT2
                    lg = scan.tile([1, BS], f32, tag="lg")
                    nc.scalar.activation(lg, cs[0:1, :], LN)
                    nc.vector.tensor_add(logz, logz, lg)

            # finish: denom = log(sum_j aT[j] * e^{end_j}) + logz
            afin = scan.tile([L, BS], f32, tag="afin")
            nc.any.tensor_scalar_mul(afin, aT, eend_sb)
            fs = scanp.tile([L, BS], f32, tag="fs")
            nc.tensor.matmul(fs, ones_sb, afin, start=True, stop=True)
            lgf = scan.tile([1, BS], f32, tag="lgf")
            nc.scalar.activation(lgf, fs[0:1, :], LN)
            nc.vector.tensor_add(den_sb, logz, lgf)
            nc.sync.dma_start(den_out[:, :], den_sb)

    return nc


def _numerator(emissions, start_transitions, end_transitions, transitions,
               tags, mask):
    maskf = mask.astype(np.float32)
    emit_gold = np.take_along_axis(
        emissions, tags[..., None].astype(np.int64), axis=2)[..., 0]
    score = start_transitions[tags[:, 0]] + emit_gold[:, 0]
    trans_gold = transitions[tags[:, :-1], tags[:, 1:]]
    score = score + np.sum((trans_gold + emit_gold[:, 1:]) * maskf[:, 1:],
                           axis=1)
    seq_ends = np.sum(mask.astype(np.int64), axis=1) - 1
    last_tags = np.take_along_axis(tags.astype(np.int64),
                                   seq_ends[:, None], axis=1)[:, 0]
    return score + end_transitions[last_tags]


def _host_denominator(emissions, start_transitions, end_transitions,
                      transitions, mask):
    # log-domain forward algorithm, numpy (fallback path only)
    Bm = emissions.shape[0]
    alpha = start_transitions[None, :] + emissions[:, 0]
    for t in range(1, emissions.shape[1]):
        x = alpha[:, :, None] + transitions[None, :, :] + \
            emissions[:, t][:, None, :]
        m = np.max(x, axis=1, keepdims=True)
        nxt = np.squeeze(m, 1) + np.log(np.sum(np.exp(x - m), axis=1))
        alpha = np.where(mask[:, t][:, None], nxt, alpha)
    x = alpha + end_transitions[None, :]
    m = np.max(x, axis=1, keepdims=True)
    return np.squeeze(m, 1) + np.log(np.sum(np.exp(x - m), axis=1))


def _run_device(x, W, b, start_transitions, end_transitions, transitions):
    from concourse.bass_utils import run_bass_kernel_spmd

    nc = _build_nc()
    wt_full = np.ascontiguousarray(W.T).astype(np.float32)          # [H, L]
    expT_m = np.exp(transitions + b[None, :]).astype(np.float32)    # [L, L]
    estart = np.exp(start_transitions + b)[:, None].astype(np.float32)
    eend = np.exp(end_transitions)[:, None].astype(np.float32)
    ones16 = np.ones((L, L), dtype=np.float32)

    in_maps = []
    for i in range(NCORES):
        xs = x[i * BS:(i + 1) * BS]                                 # [BS,T,H]
        xt = np.ascontiguousarray(xs.transpose(2, 1, 0)).reshape(H, PTS)
        in_maps.append({
            "xt": xt.astype(np.float32), "wt": wt_full, "expT": expT_m,
            "estart": estart, "eend": eend, "ones16": ones16,
        })

    res = run_bass_kernel_spmd(nc, in_maps, core_ids=list(range(NCORES)))
    results = res.results

    em_parts, den_parts = [], []
    for i in range(NCORES):
        r = results[i]
        em = np.asarray(r["em_out"]).reshape(L, T, BS).transpose(2, 1, 0)
        em_parts.append(em)                                         # [BS,T,L]
        den_parts.append(np.asarray(r["den_out"]).reshape(BS))
    emissions = np.concatenate(em_parts, axis=0)                    # [B,T,L]
    denom = np.concatenate(den_parts, axis=0)                       # [B]
    return emissions, denom


def kernel(x, W, b, start_transitions, end_transitions, transitions,
           tags, mask):
    x = np.asarray(x, dtype=np.float32)
    W = np.asarray(W, dtype=np.float32)
    b = np.asarray(b, dtype=np.float32)
    start_transitions = np.asarray(start_transitions, dtype=np.float32)
    end_transitions = np.asarray(end_transitions, dtype=np.float32)
    transitions = np.asarray(transitions, dtype=np.float32)
    tags = np.asarray(tags)
    mask = np.asarray(mask).astype(bool)

    try:
        em_dev, denom = _run_device(x, W, b, start_transitions,
                                    end_transitions, transitions)
        emissions = em_dev + b[None, None, :]
    except Exception:
        emissions = np.einsum('bth,lh->btl', x, W) + b[None, None, :]
        denom = _host_denominator(emissions, start_transitions,
                                  end_transitions, transitions, mask)

    score = _numerator(emissions, start_transitions, end_transitions,
                       transitions, tags, mask)
    llh = score - denom
    return np.float32(-np.mean(llh))

